# revision 44
# baseline (speedup 1.0000x reference)
"""NNUE (HalfKP sparse embedding + tiny MLP) Trainium2 kernel — sparse-compacted.

v5 strategy (vs v2 at 74us; v3 bitpacked the A stream, 71us):
  The kernel is an HBM-bandwidth problem: per core it streams gathered fp8
  weight rows and multiplies them against a stationary 0/1 activation matrix
  per 128-sample batch tile (DoubleRow fp8, batch on PSUM partitions, 512
  accumulator columns on the free dim), then relu -> PE transpose -> the tiny
  512->32->32->1 MLP. Pure batch data-parallel over 8 cores, no collectives.

  v5 adds, on top of v3's bitpacked A (8x smaller; expanded on DVE with
  u32-lane shift+mask into 0x40 bytes == fp8 2.0, the 2x folded into W0):

  * Pair-block dedup of the weight gather. Within a core, a key active in
    2+ of the 4 batch tiles was gathered 2-4x (1.31x average duplication).
    Keys are now classified by exact tile membership: single-tile keys go to
    per-tile exclusive blocks, two-tile keys to one of 6 pair blocks (DMA'd
    once, accumulated by both tiles), 3/4-tile keys split pair+exclusive.
    For this sparsity (~7.2k union/tile) the per-tile matmul plane count is
    unchanged (58), but the weight stream drops 15.2MB -> ~12MB/core.

  * Dual-ring weight streaming: wc chunks alternate between the SP and ACT
    HWDGE rings. A single ring paces dma_start dispatches against data
    completions (~2 in flight), which starved the stream's ramp and tail;
    two rings overlap their fixed completion latencies.

  * One SBUF tile for the whole weight stream, DMA'd in plane-range chunks;
    matmuls depend on chunk sems via AP overlap, so chunk boundaries are
    decoupled from block boundaries.
"""

import numpy as np
import ml_dtypes

B = 4096
H = 20480
D = 2 * H
NCORES = 8
BC = B // NCORES   # 512 samples per core
T = 128            # batch-tile size (PSUM partition dim)
TPC = BC // T      # 4 tiles per core
OFF = D + 1        # keyspace size per pov half (D features + 1 bias row)

bf16 = ml_dtypes.bfloat16
f8 = ml_dtypes.float8_e4m3fn
F8MAX = 240.0  # TRN FP8_EXP4 max normal is +-240 (not OCP's 448)

TRACE = False
LAST_EXEC_NS = None
LAST_RESULTS = None

_COMPILED = {}

# pair order: consumption-ordered (tile0's pairs, then new pairs per tile)
PAIRS = [(0, 1), (0, 2), (0, 3), (1, 2), (1, 3), (2, 3)]


def _prune_redundant_dma_waits(nc, mybir):
    """Drop transitively-implied waits from DMA instructions."""
    from collections import defaultdict

    f = nc.m.functions[0]
    insts = [i for b in f.blocks for i in b.instructions]

    def is_dma(i):
        return "dma" in type(i).__name__.lower()

    def wait_list(i):
        si = getattr(i, "sync_info", None)
        if si is None:
            return []
        return [
            (w.ant_name, w.wait_value)
            for w in si.on_wait
            if w.wait_mode == "sem-ge-imm" and w.wait_value is not None
        ]

    def update_list(i):
        si = getattr(i, "sync_info", None)
        if si is None:
            return []
        out = []
        for u in si.on_update:
            if u.update_mode == "sem-add-imm" and u.update_value is not None:
                out.append((u.ant_name, u.update_value))
            elif u.update_mode == "sem-inc":
                out.append((u.ant_name, 1))
            else:
                out.append((u.ant_name, None))
        return out

    sem_hist = defaultdict(list)
    poisoned = set()
    cum = defaultdict(int)
    eng_clock = {}

    def join(a, b):
        if not b:
            return a
        out = dict(a)
        for k, v in b.items():
            if out.get(k, -1) < v:
                out[k] = v
        return out

    def clock_at(sem, val):
        if sem in poisoned:
            return None
        hist = sem_hist.get(sem)
        if not hist:
            return None
        lo, hi = 0, len(hist)
        while lo < hi:
            mid = (lo + hi) // 2
            if hist[mid][0] < val:
                lo = mid + 1
            else:
                hi = mid
        if lo == len(hist):
            return None
        return hist[lo][1]

    for i in insts:
        c = {}
        eng = getattr(i, "engine", None)
        if not is_dma(i) and eng is not None and eng in eng_clock:
            c = dict(eng_clock[eng])
        for sem, val in wait_list(i):
            wc = clock_at(sem, val)
            if wc is not None:
                c = join(c, wc)
            if c.get(sem, -1) < val:
                c[sem] = val
        for sem, inc in update_list(i):
            if inc is None:
                poisoned.add(sem)
                continue
            cum[sem] += inc
            c = join(c, {sem: cum[sem]})
            sem_hist[sem].append((cum[sem], c))
        if not is_dma(i) and eng is not None:
            eng_clock[eng] = c

    n_dropped = 0
    for i in insts:
        if not is_dma(i):
            continue
        si = getattr(i, "sync_info", None)
        if si is None or len(si.on_wait) <= 1:
            continue
        kept = list(si.on_wait)
        for w in list(kept):
            if len(kept) <= 1:
                break
            if w.wait_mode != "sem-ge-imm" or w.wait_value is None:
                continue
            others = {}
            ok = True
            for o in kept:
                if o is w:
                    continue
                if o.wait_mode != "sem-ge-imm" or o.wait_value is None:
                    ok = False
                    break
                oc = clock_at(o.ant_name, o.wait_value)
                if oc is None:
                    ok = False
                    break
                others = join(others, oc)
            if ok and others.get(w.ant_name, -1) >= w.wait_value:
                kept.remove(w)
                n_dropped += 1
        if len(kept) != len(si.on_wait):
            i.sync_info = mybir.SyncInfo(on_wait=kept, on_update=list(si.on_update))
    return n_dropped


def _plan_chunks(total, lead, body, tail):
    """Split `total` k-planes into DMA chunks. All sizes even."""
    c = list(lead)
    rem = total - sum(lead) - sum(tail)
    assert rem >= 0 and rem % 2 == 0, (total, rem)
    nb = rem // body
    c += [body] * nb
    if rem - nb * body:
        c.append(rem - nb * body)
    c += list(tail)
    assert sum(c) == total
    return c


# block order in the global wc plane array = first-use order
BLOCK_ORDER = [("p", 0, 1), ("p", 0, 2), ("p", 0, 3), ("e", 0),
               ("p", 1, 2), ("p", 1, 3), ("e", 1),
               ("p", 2, 3), ("e", 2), ("e", 3)]


def _tile_blocks(t):
    """blocks a tile accumulates, in consumption (= first-use) order"""
    return [("p",) + p for p in PAIRS if t in p] + [("e", t)]


def _layout(sizes):
    """sizes: dict block -> even plane count. Returns (wc_off: block ->
    global wc plane offset, total wc planes, per-tile segment lists
    [(a_off, wc_off, nplanes)], total A planes, per-tile A plane counts)."""
    wc_off = {}
    g = 0
    for b in BLOCK_ORDER:
        wc_off[b] = g
        g += sizes[b]
    wc_total = g
    tiles = []
    apt = []
    a_off = 0
    for t in range(TPC):
        segs = []
        a0 = a_off
        for bk in _tile_blocks(t):
            segs.append((a_off, wc_off[bk], sizes[bk]))
            a_off += sizes[bk]
        apt.append(a_off - a0)
        tiles.append(segs)
    return wc_off, wc_total, tiles, a_off, apt


def _build(sizes_t):
    import concourse.bacc as bacc
    import concourse.mybir as mybir
    import concourse.tile as tile
    from concourse.bass import ts
    from concourse.masks import make_identity

    fp32 = mybir.dt.float32
    f8t = mybir.dt.float8e4
    bft = mybir.dt.bfloat16
    u8 = mybir.dt.uint8
    u32 = mybir.dt.uint32

    sizes = dict(zip(BLOCK_ORDER, sizes_t))
    wc_off, WCP, tiles, ANKP, apt = _layout(sizes)

    nc = bacc.Bacc("TRN2", target_bir_lowering=False, debug=False)

    wc = nc.dram_tensor("wc", (128, WCP, 512), f8t, kind="ExternalInput").ap()
    # bitpacked A: byte j of a plane holds samples {q*16+j : q in 0..7} in bit q
    acp = nc.dram_tensor("acp", (128, ANKP, 16), u8, kind="ExternalInput").ap()
    # host-pre-expanded A planes [0, 8): 0x00/0x40 bytes
    ac0 = nc.dram_tensor("ac0", (128, 8, 128), u8, kind="ExternalInput").ap()
    # pack[0:32, 0]=b0, [0:32, 1]=b1, [0,2]=b2, [0:32, 3:35]=W1^T, [0:32, 35]=W2
    pack = nc.dram_tensor("pack", (128, 36), fp32, kind="ExternalInput").ap()
    w0t = nc.dram_tensor("w0t", (128, 4, 32), bft, kind="ExternalInput").ap()
    out = nc.dram_tensor("out", (1, BC), fp32, kind="ExternalOutput").ap()

    relu = mybir.ActivationFunctionType.Relu
    ident_f = mybir.ActivationFunctionType.Identity
    dr = mybir.MatmulPerfMode.DoubleRow
    shl = mybir.AluOpType.logical_shift_left
    shr = mybir.AluOpType.logical_shift_right
    band = mybir.AluOpType.bitwise_and

    # 16-plane chunks: completion sems are paced by the slowest SDMA engine
    # (per-engine rate varies 21-26 GB/s run to run); small chunks keep the
    # PE's per-chunk waits under the ~3.4us HAM idle window so the clock
    # gate never re-throttles mid-run.
    wc_plan = _plan_chunks(WCP, (), 16, (8, 4))
    # packed-A chunks: planes [0,8) ship host-pre-expanded (ac0) so the PE
    # can start without waiting on DVE; the rest expand on-device
    acp_plan = [(8, apt[0]), (apt[0], apt[0] + apt[1]),
                (apt[0] + apt[1], apt[0] + apt[1] + apt[2]),
                (apt[0] + apt[1] + apt[2], ANKP)]

    with tile.TileContext(nc) as tc:
        with (
            tc.tile_pool(name="consts", bufs=1) as cp,
            tc.tile_pool(name="acts", bufs=1) as apl,
            tc.tile_pool(name="wts", bufs=1) as wp,
            tc.tile_pool(name="xs", bufs=1) as xp,
            tc.tile_pool(name="tmps", bufs=2) as tp,
            tc.tile_pool(name="psx", bufs=1, space="PSUM") as pp,
            tc.tile_pool(name="pst", bufs=1, space="PSUM") as pp2,
            tc.tile_pool(name="psm", bufs=1, space="PSUM") as pp3,
            tc.tile_pool(name="psw", bufs=1, space="PSUM") as pp4,
        ):
            # The full weight stream flows strictly in consumption order on
            # the SP HWDGE ring (a single FIFO queue keeps plane arrival
            # in-order; dispatch is ring-paced ~4 deep so chunks are kept
            # uniform). The packed-A chunks + consts ride the ACT ring.
            wcs = wp.tile([128, WCP, 512], f8t, tag="wc", name="wc_s")
            acps = apl.tile([128, ANKP, 16], u8, tag="acp", name="acp_s")
            ac_x = apl.tile([128, ANKP, 128], u8, tag="acx", name="ac_x")
            pack_s = cp.tile([128, 36], fp32, tag="pack", name="pack_s")
            w0t_s = cp.tile([128, 4, 32], bft, tag="w0t", name="w0t_s")

            nc.scalar.dma_start(ac_x[:, 0:8, :], ac0)
            wc_bounds = np.cumsum([0] + wc_plan)
            for i, n in enumerate(wc_plan):
                g = int(wc_bounds[i])
                nc.sync.dma_start(wcs[:, g : g + n, :], wc[:, g : g + n, :])
                if i < len(acp_plan):
                    lo, hi = acp_plan[i]
                    nc.scalar.dma_start(
                        acps[:, lo:hi, :], acp[:, lo:hi, :]
                    )
                elif i == len(acp_plan):
                    nc.scalar.dma_start(pack_s[:], pack)
                elif i == len(acp_plan) + 1:
                    nc.scalar.dma_start(w0t_s[:], w0t)

            # PE warm-up: ~3us of dummy matmuls (zeros x zeros into a scratch
            # PSUM bank nobody reads) so the HAM clock gate opens before the
            # first real matmul. The operand is a DVE-memset tile so warm-up
            # starts as soon as the engine preambles finish (~6.5us), well
            # before the identity build or the first weight chunk.
            dz_s = cp.tile([128, 512], bft, tag="dz", name="dz_s")
            nc.vector.memset(dz_s[:], 0)
            warm_ps = pp4.tile([128, 512], fp32, tag="warm", name="warm_ps")
            for _ in range(36):
                nc.tensor.matmul(
                    warm_ps[:, 0:128], dz_s[:, 0:128], dz_s[:, 0:128],
                    start=True, stop=True,
                )

            ident_s = cp.tile([128, 128], bft, tag="ident", name="ident_s")
            make_identity(nc, ident_s[:])

            b0_ap = pack_s[0:32, 0:1]
            b1_ap = pack_s[0:32, 1:2]
            b2_ap = pack_s[0:1, 2:3]
            w1t_ap = pack_s[0:32, 3:35]
            w2t_ap = pack_s[0:32, 35:36]

            # On-device A expansion on DVE: bytes 0x00/0x40; 0x40 == fp8e4
            # 2.0 (the 2x is folded into W0 on the host). u32 lanes process
            # 4 packed bytes at once; the per-byte shift+mask never crosses
            # byte boundaries after the 0x40404040 AND.
            ac32 = ac_x[:].bitcast(u32)
            acp32 = acps[:].bitcast(u32)
            M4 = 0x40404040
            for lo, hi in acp_plan:
                src = acp32[:, lo:hi, :]
                for q in range(8):
                    dst = ac32[:, lo:hi, q * 4 : (q + 1) * 4]
                    if q < 6:
                        nc.vector.tensor_scalar(dst, src, 6 - q, M4, shl, band)
                    elif q == 6:
                        nc.vector.tensor_scalar(dst, src, M4, None, band)
                    else:
                        nc.vector.tensor_scalar(dst, src, 1, M4, shr, band)
            ac8 = ac_x[:].bitcast(f8t)

            ys_s = xp.tile([1, BC], fp32, tag="ys", name="ys_s")

            x_chain = [None] * TPC

            # post(t) = relu -> transpose -> tiny MLP -> store for one batch
            # tile, split into three pieces interleaved into the NEXT tile's
            # matmul stream: the PE<->ACT ping-pong (relu between the MLP
            # matmuls) then hides behind DR matmuls instead of stalling the
            # PE queue.  NOTE: dma_start_transpose (to offload the PE) was
            # tried twice and regresses badly — the framework's xbar-
            # transpose-vs-SBUF-DMA deadlock guard serializes it against the
            # in-flight weight stream. PE transpose it is.
            def post_a(t):
                x_sb = xp.tile([128, 512], bft, tag=f"xsb{t % 2}", name="x_sb")
                for a in range(4):
                    nc.scalar.activation(
                        x_sb[:, ts(a, 128)], x_chain[t][:, ts(a, 128)], relu
                    )
                xt_sb = xp.tile([128, 4, 128], bft, tag=f"xt{t % 2}", name="xt_sb")
                for a in range(4):
                    xt_ps = pp2.tile(
                        [128, 128], bft, tag=f"xtp{a % 2}", name="xt_ps"
                    )
                    nc.tensor.transpose(xt_ps[:], x_sb[:, ts(a, 128)], ident_s[:])
                    nc.vector.tensor_copy(xt_sb[:, a, :], xt_ps[:])
                h0 = pp3.tile([32, 128], fp32, tag="h0", name="h0")
                for a in range(4):
                    nc.tensor.matmul(
                        h0[:],
                        w0t_s[:, a, :],
                        xt_sb[:, a, :],
                        start=(a == 0),
                        stop=(a == 3),
                    )
                h0s = tp.tile([32, 128], fp32, tag="h0s", name="h0s")
                nc.scalar.activation(h0s[:], h0[:], relu, bias=b0_ap)
                return xt_sb, h0s

            def post_b(t, h0s):
                h1 = pp3.tile([32, 128], fp32, tag="h1", name="h1")
                nc.tensor.matmul(h1[:], w1t_ap, h0s[:], start=True, stop=True)
                h1s = tp.tile([32, 128], fp32, tag="h1s", name="h1s")
                nc.scalar.activation(h1s[:], h1[:], relu, bias=b1_ap)
                return h1s

            def post_c(t, h1s):
                y_ps = pp3.tile([1, 128], fp32, tag="y", name="y_ps")
                nc.tensor.matmul(y_ps[:], w2t_ap, h1s[:], start=True, stop=True)
                nc.scalar.activation(ys_s[:, ts(t, 128)], y_ps[:], ident_f, bias=b2_ap)
                # per-tile output store: only 512B ride the final tail
                nc.scalar.dma_start(out[:, ts(t, 128)], ys_s[:, ts(t, 128)])

            # dummies emitted between the early chunks' matmul groups keep
            # the PE busy through the DMA ramp (no HAM reset, no cold runs)
            pad_after_chunk = {0: 2, 1: 2, 2: 2}

            def pad(n):
                for _ in range(n):
                    nc.tensor.matmul(
                        warm_ps[:], dz_s[:, 0:128], dz_s[:], start=True,
                        stop=True, skip_group_check=True,
                    )

            cur_chunk = 0
            for t in range(TPC):
                x_ps = pp.tile([128, 512], fp32, tag=f"x{t % 2}", name="x_ps")
                segs = tiles[t]
                nseg = len(segs)
                nbt = sum(npl // 2 for _, _, npl in segs)  # DR blocks this tile
                marks = {nbt // 4: "a", nbt // 2: "b", (3 * nbt) // 4: "c"}
                pieces = {}
                bi = 0
                for si, (a_off, w_off, npl) in enumerate(segs):
                    for nb in range(npl // 2):
                        wpl = w_off + 2 * nb
                        if t == 0:
                            ck = int(np.searchsorted(wc_bounds, wpl, "right")) - 1
                            while cur_chunk < ck:
                                pad(pad_after_chunk.get(cur_chunk, 0))
                                cur_chunk += 1
                        if t > 0 and bi in marks:
                            m = marks[bi]
                            if m == "a":
                                pieces["a"] = post_a(t - 1)
                            elif m == "b":
                                pieces["b"] = post_b(t - 1, pieces["a"][1])
                            else:
                                post_c(t - 1, pieces["b"])
                        nc.tensor.matmul(
                            x_ps[:],
                            ac8[:, a_off + 2 * nb : a_off + 2 * nb + 2, :],
                            wcs[:, wpl : wpl + 2, :],
                            start=(si == 0 and nb == 0),
                            stop=(si == nseg - 1 and nb == npl // 2 - 1),
                            perf_mode=dr,
                            skip_group_check=True,
                        )
                        bi += 1
                x_chain[t] = x_ps
            xt_h0s = post_a(TPC - 1)
            h1s = post_b(TPC - 1, xt_h0s[1])
            post_c(TPC - 1, h1s)

    _prune_redundant_dma_waits(nc, mybir)
    nc.compile()
    return nc


def _get_compiled(sizes_t):
    if sizes_t not in _COMPILED:
        _COMPILED[sizes_t] = _build(sizes_t)
    return _COMPILED[sizes_t]


def kernel(pov, white, black, Ww, bw, Wb, bb, W0, b0, W1, b1, W2, b2):
    global LAST_EXEC_NS, LAST_RESULTS
    from concourse import bass_utils

    pov = np.asarray(pov, np.float32)
    white = np.asarray(white, np.float32)
    black = np.asarray(black, np.float32)
    Ww = np.asarray(Ww, np.float32)
    Wb = np.asarray(Wb, np.float32)

    # ---- quantized combined table (row f<H: white feature; H<=f<D: black;
    # f=D: bias). Second half is the 256-half-swapped copy for pov=0 samples.
    Wf = np.empty((OFF, 512), np.float32)
    Wf[:H, :256] = Ww[:, :H].T
    Wf[H:D, :256] = Ww[:, H:].T
    Wf[:H, 256:] = Wb[:, H:].T
    Wf[H:D, 256:] = Wb[:, :H].T
    Wf[D, :256] = np.asarray(bw, np.float32)
    Wf[D, 256:] = np.asarray(bb, np.float32)
    colmax = np.abs(Wf).max(axis=0)
    s256 = np.maximum(np.maximum(colmax[:256], colmax[256:]) / F8MAX, 1e-30)
    s512 = np.concatenate([s256, s256])
    Wq = (Wf / s512[None, :]).astype(f8)
    perm = np.concatenate([np.arange(256, 512), np.arange(256)])
    table = np.concatenate([Wq, Wq[:, perm]], axis=0)  # [2*OFF, 512]

    # ---- per-sample keys, pov-sorted sample order
    pov1 = pov.reshape(-1) > 0.5
    order = np.argsort(np.where(pov1, 0, 1), kind="stable")
    pos = np.empty(B, np.int64)
    pos[order] = np.arange(B)
    povoff = np.where(pov1, 0, OFF).astype(np.int64)

    wnz_b, wnz_f = np.nonzero(white > 0.5)
    bnz_b, bnz_f = np.nonzero(black > 0.5)
    allk = np.concatenate(
        [
            wnz_f + povoff[wnz_b],
            (bnz_f + H) + povoff[bnz_b],
            D + povoff,
        ]
    )
    allb = np.concatenate([wnz_b, bnz_b, np.arange(B)])
    allpos = pos[allb]
    tile_id = allpos // T
    col = (allpos % T).astype(np.int64)
    o = np.argsort(tile_id, kind="stable")
    allk, col, tile_id = allk[o], col[o], tile_id[o]
    bounds = np.searchsorted(tile_id, np.arange(B // T + 1))

    # ---- per-core pair-block classification
    NTILES = B // T
    per_tile = []
    for t in range(NTILES):
        lo, hi = bounds[t], bounds[t + 1]
        ku, inv = np.unique(allk[lo:hi], return_inverse=True)
        per_tile.append((ku, inv, col[lo:hi]))

    # For each core: membership mask per key over its 4 tiles, then assign
    # every key to blocks: |m|=1 -> excl; |m|=2 -> pair; |m|=3 -> pair of
    # first two + excl of third; |m|=4 -> pair01 + pair23.
    pair_idx = {p: i for i, p in enumerate(PAIRS)}
    core_blocks = []   # per core: dict block -> list of keys
    for c in range(NCORES):
        memb = {}
        for i in range(TPC):
            ku = per_tile[c * TPC + i][0]
            for k in ku.tolist():
                memb[k] = memb.get(k, 0) | (1 << i)
        blocks = {("e", t): [] for t in range(TPC)}
        blocks.update({("p",) + p: [] for p in PAIRS})
        # two passes: place 1/2-tile keys, then balance 3/4-tile keys into
        # the least-filled blocks (padding is per-slot max across cores)
        multi = []
        for k, m in memb.items():
            ts_in = [t for t in range(TPC) if m >> t & 1]
            if len(ts_in) == 1:
                blocks[("e", ts_in[0])].append(k)
            elif len(ts_in) == 2:
                blocks[("p",) + tuple(ts_in)].append(k)
            else:
                multi.append((k, ts_in))
        for k, ts_in in multi:
            if len(ts_in) == 3:
                a, b_, c2 = ts_in
                opts = [((a, b_), c2), ((a, c2), b_), ((b_, c2), a)]
                pr, ex = min(
                    opts,
                    key=lambda o: (
                        len(blocks[("p",) + o[0]]),
                        len(blocks[("e", o[1])]),
                    ),
                )
                blocks[("p",) + pr].append(k)
                blocks[("e", ex)].append(k)
            else:
                pairings = [
                    ((0, 1), (2, 3)),
                    ((0, 2), (1, 3)),
                    ((0, 3), (1, 2)),
                ]
                p1, p2 = min(
                    pairings,
                    key=lambda o: len(blocks[("p",) + o[0]])
                    + len(blocks[("p",) + o[1]]),
                )
                blocks[("p",) + p1].append(k)
                blocks[("p",) + p2].append(k)
        core_blocks.append(blocks)

    sizes = {}
    for bk in BLOCK_ORDER:
        mx = max(len(b[bk]) for b in core_blocks)
        sizes[bk] = -(-mx // 256) * 2   # even plane count, >= rows/128
    sizes_t = tuple(sizes[bk] for bk in BLOCK_ORDER)

    wc_off, WCP, tiles_lay, ANKP, apt = _layout(sizes)

    wc_all = np.zeros((NCORES, 128, WCP, 512), f8)
    acp_all = np.zeros((NCORES, 128, ANKP, 16), np.uint8)
    for c in range(NCORES):
        blocks = core_blocks[c]
        for bk, keys in blocks.items():
            keys.sort()
            npl = sizes[bk]
            goff = wc_off[bk]
            n = len(keys)
            # block row r -> global plane goff + r//128, partition r%128
            tmp = np.zeros((npl * 128, 512), f8)
            tmp[:n] = table[np.asarray(keys, np.int64)]
            wc_all[c, :, goff : goff + npl, :] = (
                tmp.reshape(npl, 128, 512).transpose(1, 0, 2)
            )
        # A bits per tile: row in the tile's A section = seg a_off*128 + r
        for ti in range(TPC):
            ku, inv, cols = per_tile[c * TPC + ti]
            key2a = {}
            for (a_off, _w, _n), bk in zip(tiles_lay[ti], _tile_blocks(ti)):
                base = a_off * 128
                for r, k in enumerate(blocks[bk]):
                    key2a[k] = base + r
            rows_a = np.array([key2a[k] for k in ku.tolist()], np.int64)
            occ_rows = rows_a[inv]        # A row per occurrence
            pl = occ_rows // 128
            pt = occ_rows % 128
            np.bitwise_or.at(
                acp_all[c],
                (pt, pl, cols % 16),
                (1 << (cols // 16)).astype(np.uint8),
            )

    # ---- MLP constants; fold dequant scales and the A-matrix 2.0 into W0
    W0p = np.asarray(W0, np.float32) * (s512[None, :] * 0.5)
    w0t_dev = np.ascontiguousarray(
        W0p.T.reshape(4, 128, 32).transpose(1, 0, 2).astype(bf16)
    )
    pack = np.zeros((128, 36), np.float32)
    pack[0:32, 0] = np.asarray(b0, np.float32)
    pack[0:32, 1] = np.asarray(b1, np.float32)
    pack[0, 2] = float(np.asarray(b2).reshape(-1)[0])
    pack[0:32, 3:35] = np.asarray(W1, np.float32).T
    pack[0:32, 35] = np.asarray(W2, np.float32).reshape(32)

    # host-pre-expanded first 8 A planes (sample s = q*16+j <-> bit q, byte j)
    qs = np.arange(8, dtype=np.uint8)
    bits = (acp_all[:, :, :8, :, None] >> qs[None, None, None, None, :]) & 1
    ac0_all = np.ascontiguousarray(
        (bits.transpose(0, 1, 2, 4, 3) * np.uint8(0x40)).reshape(NCORES, 128, 8, 128)
    )

    in_maps = []
    for c in range(NCORES):
        in_maps.append(
            {
                "wc": wc_all[c],
                "acp": acp_all[c],
                "ac0": ac0_all[c],
                "pack": pack,
                "w0t": w0t_dev,
            }
        )

    nc = _get_compiled(sizes_t)
    for attempt in range(3):
        res = bass_utils.run_bass_kernel_spmd(
            nc, in_maps, core_ids=list(range(NCORES)), trace=TRACE
        )
        y_sorted = np.concatenate(
            [res.results[c]["out"].reshape(BC) for c in range(NCORES)]
        )
        if np.isfinite(y_sorted).all():
            break
    LAST_EXEC_NS = res.exec_time_ns
    LAST_RESULTS = res

    y = np.empty((B, 1), np.float32)
    y[order, 0] = y_sorted
    return y


# revision 48
# speedup vs baseline: 1.0429x; 1.0429x over previous
"""NNUE (HalfKP sparse embedding + tiny MLP) Trainium2 kernel — sparse-compacted.

v5 strategy (vs v2 at 74us; v3 bitpacked the A stream, 71us):
  The kernel is an HBM-bandwidth problem: per core it streams gathered fp8
  weight rows and multiplies them against a stationary 0/1 activation matrix
  per 128-sample batch tile (DoubleRow fp8, batch on PSUM partitions, 512
  accumulator columns on the free dim), then relu -> PE transpose -> the tiny
  512->32->32->1 MLP. Pure batch data-parallel over 8 cores, no collectives.

  v5 adds, on top of v3's bitpacked A (8x smaller; expanded on DVE with
  u32-lane shift+mask into 0x40 bytes == fp8 2.0, the 2x folded into W0):

  * Pair-block dedup of the weight gather. Within a core, a key active in
    2+ of the 4 batch tiles was gathered 2-4x (1.31x average duplication).
    Keys are now classified by exact tile membership: single-tile keys go to
    per-tile exclusive blocks, two-tile keys to one of 6 pair blocks (DMA'd
    once, accumulated by both tiles), 3/4-tile keys split pair+exclusive.
    For this sparsity (~7.2k union/tile) the per-tile matmul plane count is
    unchanged (58), but the weight stream drops 15.2MB -> ~12MB/core.

  * Dual-ring weight streaming: wc chunks alternate between the SP and ACT
    HWDGE rings. A single ring paces dma_start dispatches against data
    completions (~2 in flight), which starved the stream's ramp and tail;
    two rings overlap their fixed completion latencies.

  * One SBUF tile for the whole weight stream, DMA'd in plane-range chunks;
    matmuls depend on chunk sems via AP overlap, so chunk boundaries are
    decoupled from block boundaries.
"""

import numpy as np
import ml_dtypes

B = 4096
H = 20480
D = 2 * H
NCORES = 8
BC = B // NCORES   # 512 samples per core
T = 128            # batch-tile size (PSUM partition dim)
TPC = BC // T      # 4 tiles per core
OFF = D + 1        # keyspace size per pov half (D features + 1 bias row)

bf16 = ml_dtypes.bfloat16
f8 = ml_dtypes.float8_e4m3fn
F8MAX = 240.0  # TRN FP8_EXP4 max normal is +-240 (not OCP's 448)

TRACE = False
LAST_EXEC_NS = None
LAST_RESULTS = None

_COMPILED = {}

# pair order: consumption-ordered (tile0's pairs, then new pairs per tile)
PAIRS = [(0, 1), (0, 2), (0, 3), (1, 2), (1, 3), (2, 3)]


def _prune_redundant_dma_waits(nc, mybir):
    """Drop transitively-implied waits from DMA instructions."""
    from collections import defaultdict

    f = nc.m.functions[0]
    insts = [i for b in f.blocks for i in b.instructions]

    def is_dma(i):
        return "dma" in type(i).__name__.lower()

    def wait_list(i):
        si = getattr(i, "sync_info", None)
        if si is None:
            return []
        return [
            (w.ant_name, w.wait_value)
            for w in si.on_wait
            if w.wait_mode == "sem-ge-imm" and w.wait_value is not None
        ]

    def update_list(i):
        si = getattr(i, "sync_info", None)
        if si is None:
            return []
        out = []
        for u in si.on_update:
            if u.update_mode == "sem-add-imm" and u.update_value is not None:
                out.append((u.ant_name, u.update_value))
            elif u.update_mode == "sem-inc":
                out.append((u.ant_name, 1))
            else:
                out.append((u.ant_name, None))
        return out

    sem_hist = defaultdict(list)
    poisoned = set()
    cum = defaultdict(int)
    eng_clock = {}

    def join(a, b):
        if not b:
            return a
        out = dict(a)
        for k, v in b.items():
            if out.get(k, -1) < v:
                out[k] = v
        return out

    def clock_at(sem, val):
        if sem in poisoned:
            return None
        hist = sem_hist.get(sem)
        if not hist:
            return None
        lo, hi = 0, len(hist)
        while lo < hi:
            mid = (lo + hi) // 2
            if hist[mid][0] < val:
                lo = mid + 1
            else:
                hi = mid
        if lo == len(hist):
            return None
        return hist[lo][1]

    for i in insts:
        c = {}
        eng = getattr(i, "engine", None)
        if not is_dma(i) and eng is not None and eng in eng_clock:
            c = dict(eng_clock[eng])
        for sem, val in wait_list(i):
            wc = clock_at(sem, val)
            if wc is not None:
                c = join(c, wc)
            if c.get(sem, -1) < val:
                c[sem] = val
        for sem, inc in update_list(i):
            if inc is None:
                poisoned.add(sem)
                continue
            cum[sem] += inc
            c = join(c, {sem: cum[sem]})
            sem_hist[sem].append((cum[sem], c))
        if not is_dma(i) and eng is not None:
            eng_clock[eng] = c

    n_dropped = 0
    for i in insts:
        if not is_dma(i):
            continue
        si = getattr(i, "sync_info", None)
        if si is None or len(si.on_wait) <= 1:
            continue
        kept = list(si.on_wait)
        for w in list(kept):
            if len(kept) <= 1:
                break
            if w.wait_mode != "sem-ge-imm" or w.wait_value is None:
                continue
            others = {}
            ok = True
            for o in kept:
                if o is w:
                    continue
                if o.wait_mode != "sem-ge-imm" or o.wait_value is None:
                    ok = False
                    break
                oc = clock_at(o.ant_name, o.wait_value)
                if oc is None:
                    ok = False
                    break
                others = join(others, oc)
            if ok and others.get(w.ant_name, -1) >= w.wait_value:
                kept.remove(w)
                n_dropped += 1
        if len(kept) != len(si.on_wait):
            i.sync_info = mybir.SyncInfo(on_wait=kept, on_update=list(si.on_update))
    return n_dropped


def _plan_chunks(total, lead, body, tail):
    """Split `total` k-planes into DMA chunks. All sizes even."""
    c = list(lead)
    rem = total - sum(lead) - sum(tail)
    assert rem >= 0 and rem % 2 == 0, (total, rem)
    nb = rem // body
    c += [body] * nb
    if rem - nb * body:
        c.append(rem - nb * body)
    c += list(tail)
    assert sum(c) == total
    return c


# block order in the global wc plane array = first-use order
BLOCK_ORDER = [("p", 0, 1), ("p", 0, 2), ("p", 0, 3), ("e", 0),
               ("p", 1, 2), ("p", 1, 3), ("e", 1),
               ("p", 2, 3), ("e", 2), ("e", 3)]


def _tile_blocks(t):
    """blocks a tile accumulates, in consumption (= first-use) order"""
    return [("p",) + p for p in PAIRS if t in p] + [("e", t)]


def _layout(sizes):
    """sizes: dict block -> even plane count. Returns (wc_off: block ->
    global wc plane offset, total wc planes, per-tile segment lists
    [(a_off, wc_off, nplanes)], total A planes, per-tile A plane counts)."""
    wc_off = {}
    g = 0
    for b in BLOCK_ORDER:
        wc_off[b] = g
        g += sizes[b]
    wc_total = g
    tiles = []
    apt = []
    a_off = 0
    for t in range(TPC):
        segs = []
        a0 = a_off
        for bk in _tile_blocks(t):
            segs.append((a_off, wc_off[bk], sizes[bk]))
            a_off += sizes[bk]
        apt.append(a_off - a0)
        tiles.append(segs)
    return wc_off, wc_total, tiles, a_off, apt


def _build(sizes_t):
    import concourse.bacc as bacc
    import concourse.mybir as mybir
    import concourse.tile as tile
    from concourse.bass import ts
    from concourse.masks import make_identity

    fp32 = mybir.dt.float32
    f8t = mybir.dt.float8e4
    bft = mybir.dt.bfloat16
    u8 = mybir.dt.uint8
    u32 = mybir.dt.uint32

    sizes = dict(zip(BLOCK_ORDER, sizes_t))
    wc_off, WCP, tiles, ANKP, apt = _layout(sizes)

    nc = bacc.Bacc("TRN2", target_bir_lowering=False, debug=False)

    wc = nc.dram_tensor("wc", (128, WCP, 512), f8t, kind="ExternalInput").ap()
    # bitpacked A: byte j of a plane holds samples {q*16+j : q in 0..7} in bit q
    acp = nc.dram_tensor("acp", (128, ANKP, 16), u8, kind="ExternalInput").ap()
    # host-pre-expanded A planes [0, 8): 0x00/0x40 bytes
    ac0 = nc.dram_tensor("ac0", (128, 8, 128), u8, kind="ExternalInput").ap()
    # pack[0:32, 0]=b0, [0:32, 1]=b1, [0,2]=b2, [0:32, 3:35]=W1^T, [0:32, 35]=W2
    pack = nc.dram_tensor("pack", (128, 36), fp32, kind="ExternalInput").ap()
    w0t = nc.dram_tensor("w0t", (128, 4, 32), bft, kind="ExternalInput").ap()
    out = nc.dram_tensor("out", (1, BC), fp32, kind="ExternalOutput").ap()

    relu = mybir.ActivationFunctionType.Relu
    ident_f = mybir.ActivationFunctionType.Identity
    dr = mybir.MatmulPerfMode.DoubleRow
    shl = mybir.AluOpType.logical_shift_left
    shr = mybir.AluOpType.logical_shift_right
    band = mybir.AluOpType.bitwise_and

    # 16-plane chunks: completion sems are paced by the slowest SDMA engine
    # (per-engine rate varies 21-26 GB/s run to run); small chunks keep the
    # PE's per-chunk waits under the ~3.4us HAM idle window so the clock
    # gate never re-throttles mid-run.
    wc_plan = _plan_chunks(WCP, (8,), 16, (8, 4))
    # packed-A chunks: planes [0,8) ship host-pre-expanded (ac0) so the PE
    # can start without waiting on DVE; the rest expand on-device
    acp_plan = [(8, apt[0]), (apt[0], apt[0] + apt[1]),
                (apt[0] + apt[1], apt[0] + apt[1] + apt[2]),
                (apt[0] + apt[1] + apt[2], ANKP)]

    with tile.TileContext(nc) as tc:
        with (
            tc.tile_pool(name="consts", bufs=1) as cp,
            tc.tile_pool(name="acts", bufs=1) as apl,
            tc.tile_pool(name="wts", bufs=1) as wp,
            tc.tile_pool(name="xs", bufs=1) as xp,
            tc.tile_pool(name="tmps", bufs=2) as tp,
            tc.tile_pool(name="psx", bufs=1, space="PSUM") as pp,
            tc.tile_pool(name="pst", bufs=1, space="PSUM") as pp2,
            tc.tile_pool(name="psm", bufs=1, space="PSUM") as pp3,
            tc.tile_pool(name="psw", bufs=1, space="PSUM") as pp4,
        ):
            # The full weight stream flows strictly in consumption order on
            # the SP HWDGE ring (a single FIFO queue keeps plane arrival
            # in-order; dispatch is ring-paced ~4 deep so chunks are kept
            # uniform). The packed-A chunks + consts ride the ACT ring.
            wcs = wp.tile([128, WCP, 512], f8t, tag="wc", name="wc_s")
            acps = apl.tile([128, ANKP, 16], u8, tag="acp", name="acp_s")
            ac_x = apl.tile([128, ANKP, 128], u8, tag="acx", name="ac_x")
            pack_s = cp.tile([128, 36], fp32, tag="pack", name="pack_s")
            w0t_s = cp.tile([128, 4, 32], bft, tag="w0t", name="w0t_s")

            nc.scalar.dma_start(ac_x[:, 0:8, :], ac0)
            wc_bounds = np.cumsum([0] + wc_plan)
            for i, n in enumerate(wc_plan):
                g = int(wc_bounds[i])
                nc.sync.dma_start(wcs[:, g : g + n, :], wc[:, g : g + n, :])
                if i < len(acp_plan):
                    lo, hi = acp_plan[i]
                    nc.scalar.dma_start(
                        acps[:, lo:hi, :], acp[:, lo:hi, :]
                    )
                elif i == len(acp_plan):
                    nc.scalar.dma_start(pack_s[:], pack)
                elif i == len(acp_plan) + 1:
                    nc.scalar.dma_start(w0t_s[:], w0t)

            # PE warm-up: ~3us of dummy matmuls (zeros x zeros into a scratch
            # PSUM bank nobody reads) so the HAM clock gate opens before the
            # first real matmul. The operand is a DVE-memset tile so warm-up
            # starts as soon as the engine preambles finish (~6.5us), well
            # before the identity build or the first weight chunk.
            dz_s = cp.tile([128, 512], bft, tag="dz", name="dz_s")
            nc.vector.memset(dz_s[:], 0)
            warm_ps = pp4.tile([128, 512], fp32, tag="warm", name="warm_ps")
            for _ in range(36):
                nc.tensor.matmul(
                    warm_ps[:, 0:128], dz_s[:, 0:128], dz_s[:, 0:128],
                    start=True, stop=True,
                )

            ident_s = cp.tile([128, 128], bft, tag="ident", name="ident_s")
            make_identity(nc, ident_s[:])

            b0_ap = pack_s[0:32, 0:1]
            b1_ap = pack_s[0:32, 1:2]
            b2_ap = pack_s[0:1, 2:3]
            w1t_ap = pack_s[0:32, 3:35]
            w2t_ap = pack_s[0:32, 35:36]

            # On-device A expansion on DVE: bytes 0x00/0x40; 0x40 == fp8e4
            # 2.0 (the 2x is folded into W0 on the host). u32 lanes process
            # 4 packed bytes at once; the per-byte shift+mask never crosses
            # byte boundaries after the 0x40404040 AND.
            ac32 = ac_x[:].bitcast(u32)
            acp32 = acps[:].bitcast(u32)
            M4 = 0x40404040
            for lo, hi in acp_plan:
                src = acp32[:, lo:hi, :]
                for q in range(8):
                    dst = ac32[:, lo:hi, q * 4 : (q + 1) * 4]
                    if q < 6:
                        nc.vector.tensor_scalar(dst, src, 6 - q, M4, shl, band)
                    elif q == 6:
                        nc.vector.tensor_scalar(dst, src, M4, None, band)
                    else:
                        nc.vector.tensor_scalar(dst, src, 1, M4, shr, band)
            ac8 = ac_x[:].bitcast(f8t)

            ys_s = xp.tile([1, BC], fp32, tag="ys", name="ys_s")

            x_chain = [None] * TPC
            # transposed relu(x) for ALL tiles, laid out [128, a-chunk, tile,
            # 128] so the whole-batch MLP runs as 6 matmuls at the end
            # instead of 24 tiny per-tile ones (each tiny MM costs a full
            # ~220ns PE pipeline slot regardless of size).
            xt_all = xp.tile([128, 4, TPC, 128], bft, tag="xta", name="xt_all")

            # per-tile epilogue: relu + PE transpose into xt_all, split into
            # two pieces interleaved into the NEXT tile's matmul stream.
            # NOTE: dma_start_transpose (to offload the PE) was tried twice
            # and regresses badly — the framework's xbar-transpose-vs-SBUF-
            # DMA deadlock guard serializes it against the in-flight weight
            # stream. PE transpose it is.
            def post_a(t):
                x_sb = xp.tile([128, 512], bft, tag=f"xsb{t % 2}", name="x_sb")
                for a in range(4):
                    nc.scalar.activation(
                        x_sb[:, ts(a, 128)], x_chain[t][:, ts(a, 128)], relu
                    )
                for a in range(2):
                    xt_ps = pp2.tile(
                        [128, 128], bft, tag=f"xtp{a % 2}", name="xt_ps"
                    )
                    nc.tensor.transpose(xt_ps[:], x_sb[:, ts(a, 128)], ident_s[:])
                    nc.vector.tensor_copy(xt_all[:, a, t, :], xt_ps[:])
                return x_sb

            def post_b(t, x_sb):
                for a in range(2, 4):
                    xt_ps = pp2.tile(
                        [128, 128], bft, tag=f"xtp{a % 2}", name="xt_ps"
                    )
                    nc.tensor.transpose(xt_ps[:], x_sb[:, ts(a, 128)], ident_s[:])
                    nc.vector.tensor_copy(xt_all[:, a, t, :], xt_ps[:])

            def mlp_all():
                h0 = pp3.tile([32, 512], fp32, tag="h0", name="h0")
                for a in range(4):
                    nc.tensor.matmul(
                        h0[:],
                        w0t_s[:, a, :],
                        xt_all[:, a, :, :],
                        start=(a == 0),
                        stop=(a == 3),
                    )
                h0s = tp.tile([32, 512], fp32, tag="h0s", name="h0s")
                nc.scalar.activation(h0s[:], h0[:], relu, bias=b0_ap)
                h1 = pp3.tile([32, 512], fp32, tag="h1", name="h1")
                nc.tensor.matmul(h1[:], w1t_ap, h0s[:], start=True, stop=True)
                h1s = tp.tile([32, 512], fp32, tag="h1s", name="h1s")
                nc.scalar.activation(h1s[:], h1[:], relu, bias=b1_ap)
                y_ps = pp3.tile([1, 512], fp32, tag="y", name="y_ps")
                nc.tensor.matmul(y_ps[:], w2t_ap, h1s[:], start=True, stop=True)
                nc.scalar.activation(ys_s[:], y_ps[:], ident_f, bias=b2_ap)
                nc.scalar.dma_start(out, ys_s[:])

            # dummies emitted between the early chunks' matmul groups keep
            # the PE busy through the DMA ramp (no HAM reset, no cold runs)
            pad_after_chunk = {0: 2, 1: 2, 2: 2}

            def pad(n):
                for _ in range(n):
                    nc.tensor.matmul(
                        warm_ps[:], dz_s[:, 0:128], dz_s[:], start=True,
                        stop=True, skip_group_check=True,
                    )

            cur_chunk = 0
            for t in range(TPC):
                x_ps = pp.tile([128, 512], fp32, tag=f"x{t % 2}", name="x_ps")
                segs = tiles[t]
                nseg = len(segs)
                nbt = sum(npl // 2 for _, _, npl in segs)  # DR blocks this tile
                marks = {nbt // 3: "a", (2 * nbt) // 3: "b"}
                pieces = {}
                bi = 0
                for si, (a_off, w_off, npl) in enumerate(segs):
                    for nb in range(npl // 2):
                        wpl = w_off + 2 * nb
                        if t == 0:
                            ck = int(np.searchsorted(wc_bounds, wpl, "right")) - 1
                            while cur_chunk < ck:
                                pad(pad_after_chunk.get(cur_chunk, 0))
                                cur_chunk += 1
                        if t > 0 and bi in marks:
                            if marks[bi] == "a":
                                pieces["a"] = post_a(t - 1)
                            else:
                                post_b(t - 1, pieces["a"])
                        nc.tensor.matmul(
                            x_ps[:],
                            ac8[:, a_off + 2 * nb : a_off + 2 * nb + 2, :],
                            wcs[:, wpl : wpl + 2, :],
                            start=(si == 0 and nb == 0),
                            stop=(si == nseg - 1 and nb == npl // 2 - 1),
                            perf_mode=dr,
                            skip_group_check=True,
                        )
                        bi += 1
                x_chain[t] = x_ps
            x_sb_last = post_a(TPC - 1)
            post_b(TPC - 1, x_sb_last)
            mlp_all()

    _prune_redundant_dma_waits(nc, mybir)
    nc.compile()
    return nc


def _get_compiled(sizes_t):
    if sizes_t not in _COMPILED:
        _COMPILED[sizes_t] = _build(sizes_t)
    return _COMPILED[sizes_t]


def kernel(pov, white, black, Ww, bw, Wb, bb, W0, b0, W1, b1, W2, b2):
    global LAST_EXEC_NS, LAST_RESULTS
    from concourse import bass_utils

    pov = np.asarray(pov, np.float32)
    white = np.asarray(white, np.float32)
    black = np.asarray(black, np.float32)
    Ww = np.asarray(Ww, np.float32)
    Wb = np.asarray(Wb, np.float32)

    # ---- quantized combined table (row f<H: white feature; H<=f<D: black;
    # f=D: bias). Second half is the 256-half-swapped copy for pov=0 samples.
    Wf = np.empty((OFF, 512), np.float32)
    Wf[:H, :256] = Ww[:, :H].T
    Wf[H:D, :256] = Ww[:, H:].T
    Wf[:H, 256:] = Wb[:, H:].T
    Wf[H:D, 256:] = Wb[:, :H].T
    Wf[D, :256] = np.asarray(bw, np.float32)
    Wf[D, 256:] = np.asarray(bb, np.float32)
    colmax = np.abs(Wf).max(axis=0)
    s256 = np.maximum(np.maximum(colmax[:256], colmax[256:]) / F8MAX, 1e-30)
    s512 = np.concatenate([s256, s256])
    Wq = (Wf / s512[None, :]).astype(f8)
    perm = np.concatenate([np.arange(256, 512), np.arange(256)])
    table = np.concatenate([Wq, Wq[:, perm]], axis=0)  # [2*OFF, 512]

    # ---- per-sample keys, pov-sorted sample order
    pov1 = pov.reshape(-1) > 0.5
    order = np.argsort(np.where(pov1, 0, 1), kind="stable")
    pos = np.empty(B, np.int64)
    pos[order] = np.arange(B)
    povoff = np.where(pov1, 0, OFF).astype(np.int64)

    wnz_b, wnz_f = np.nonzero(white > 0.5)
    bnz_b, bnz_f = np.nonzero(black > 0.5)
    allk = np.concatenate(
        [
            wnz_f + povoff[wnz_b],
            (bnz_f + H) + povoff[bnz_b],
            D + povoff,
        ]
    )
    allb = np.concatenate([wnz_b, bnz_b, np.arange(B)])
    allpos = pos[allb]
    tile_id = allpos // T
    col = (allpos % T).astype(np.int64)
    o = np.argsort(tile_id, kind="stable")
    allk, col, tile_id = allk[o], col[o], tile_id[o]
    bounds = np.searchsorted(tile_id, np.arange(B // T + 1))

    # ---- per-core pair-block classification
    NTILES = B // T
    per_tile = []
    for t in range(NTILES):
        lo, hi = bounds[t], bounds[t + 1]
        ku, inv = np.unique(allk[lo:hi], return_inverse=True)
        per_tile.append((ku, inv, col[lo:hi]))

    # For each core: membership mask per key over its 4 tiles, then assign
    # every key to blocks: |m|=1 -> excl; |m|=2 -> pair; |m|=3 -> pair of
    # first two + excl of third; |m|=4 -> pair01 + pair23.
    pair_idx = {p: i for i, p in enumerate(PAIRS)}
    core_blocks = []   # per core: dict block -> list of keys
    for c in range(NCORES):
        memb = {}
        for i in range(TPC):
            ku = per_tile[c * TPC + i][0]
            for k in ku.tolist():
                memb[k] = memb.get(k, 0) | (1 << i)
        blocks = {("e", t): [] for t in range(TPC)}
        blocks.update({("p",) + p: [] for p in PAIRS})
        # two passes: place 1/2-tile keys, then balance 3/4-tile keys into
        # the least-filled blocks (padding is per-slot max across cores)
        multi = []
        for k, m in memb.items():
            ts_in = [t for t in range(TPC) if m >> t & 1]
            if len(ts_in) == 1:
                blocks[("e", ts_in[0])].append(k)
            elif len(ts_in) == 2:
                blocks[("p",) + tuple(ts_in)].append(k)
            else:
                multi.append((k, ts_in))
        def planes(n):
            return -(-(n + 1) // 256) * 2   # planes if one more key added

        for k, ts_in in multi:
            if len(ts_in) == 3:
                a, b_, c2 = ts_in
                opts = [((a, b_), c2), ((a, c2), b_), ((b_, c2), a)]
                pr, ex = min(
                    opts,
                    key=lambda o: (
                        planes(len(blocks[("p",) + o[0]]))
                        + planes(len(blocks[("e", o[1])])),
                        len(blocks[("p",) + o[0]]),
                        len(blocks[("e", o[1])]),
                    ),
                )
                blocks[("p",) + pr].append(k)
                blocks[("e", ex)].append(k)
            else:
                pairings = [
                    ((0, 1), (2, 3)),
                    ((0, 2), (1, 3)),
                    ((0, 3), (1, 2)),
                ]
                p1, p2 = min(
                    pairings,
                    key=lambda o: (
                        planes(len(blocks[("p",) + o[0]]))
                        + planes(len(blocks[("p",) + o[1]])),
                        len(blocks[("p",) + o[0]])
                        + len(blocks[("p",) + o[1]]),
                    ),
                )
                blocks[("p",) + p1].append(k)
                blocks[("p",) + p2].append(k)
        core_blocks.append(blocks)

    sizes = {}
    for bk in BLOCK_ORDER:
        mx = max(len(b[bk]) for b in core_blocks)
        sizes[bk] = -(-mx // 256) * 2   # even plane count, >= rows/128
    sizes_t = tuple(sizes[bk] for bk in BLOCK_ORDER)

    wc_off, WCP, tiles_lay, ANKP, apt = _layout(sizes)

    wc_all = np.zeros((NCORES, 128, WCP, 512), f8)
    acp_all = np.zeros((NCORES, 128, ANKP, 16), np.uint8)
    for c in range(NCORES):
        blocks = core_blocks[c]
        for bk, keys in blocks.items():
            keys.sort()
            npl = sizes[bk]
            goff = wc_off[bk]
            n = len(keys)
            # block row r -> global plane goff + r//128, partition r%128
            tmp = np.zeros((npl * 128, 512), f8)
            tmp[:n] = table[np.asarray(keys, np.int64)]
            wc_all[c, :, goff : goff + npl, :] = (
                tmp.reshape(npl, 128, 512).transpose(1, 0, 2)
            )
        # A bits per tile: row in the tile's A section = seg a_off*128 + r
        for ti in range(TPC):
            ku, inv, cols = per_tile[c * TPC + ti]
            key2a = {}
            for (a_off, _w, _n), bk in zip(tiles_lay[ti], _tile_blocks(ti)):
                base = a_off * 128
                for r, k in enumerate(blocks[bk]):
                    key2a[k] = base + r
            rows_a = np.array([key2a[k] for k in ku.tolist()], np.int64)
            occ_rows = rows_a[inv]        # A row per occurrence
            pl = occ_rows // 128
            pt = occ_rows % 128
            np.bitwise_or.at(
                acp_all[c],
                (pt, pl, cols % 16),
                (1 << (cols // 16)).astype(np.uint8),
            )

    # ---- MLP constants; fold dequant scales and the A-matrix 2.0 into W0
    W0p = np.asarray(W0, np.float32) * (s512[None, :] * 0.5)
    w0t_dev = np.ascontiguousarray(
        W0p.T.reshape(4, 128, 32).transpose(1, 0, 2).astype(bf16)
    )
    pack = np.zeros((128, 36), np.float32)
    pack[0:32, 0] = np.asarray(b0, np.float32)
    pack[0:32, 1] = np.asarray(b1, np.float32)
    pack[0, 2] = float(np.asarray(b2).reshape(-1)[0])
    pack[0:32, 3:35] = np.asarray(W1, np.float32).T
    pack[0:32, 35] = np.asarray(W2, np.float32).reshape(32)

    # host-pre-expanded first 8 A planes (sample s = q*16+j <-> bit q, byte j)
    qs = np.arange(8, dtype=np.uint8)
    bits = (acp_all[:, :, :8, :, None] >> qs[None, None, None, None, :]) & 1
    ac0_all = np.ascontiguousarray(
        (bits.transpose(0, 1, 2, 4, 3) * np.uint8(0x40)).reshape(NCORES, 128, 8, 128)
    )

    in_maps = []
    for c in range(NCORES):
        in_maps.append(
            {
                "wc": wc_all[c],
                "acp": acp_all[c],
                "ac0": ac0_all[c],
                "pack": pack,
                "w0t": w0t_dev,
            }
        )

    nc = _get_compiled(sizes_t)
    for attempt in range(3):
        res = bass_utils.run_bass_kernel_spmd(
            nc, in_maps, core_ids=list(range(NCORES)), trace=TRACE
        )
        y_sorted = np.concatenate(
            [res.results[c]["out"].reshape(BC) for c in range(NCORES)]
        )
        if np.isfinite(y_sorted).all():
            break
    LAST_EXEC_NS = res.exec_time_ns
    LAST_RESULTS = res

    y = np.empty((B, 1), np.float32)
    y[order, 0] = y_sorted
    return y
